# revision 7
# baseline (speedup 1.0000x reference)
"""Bahdanau attention on 8 TRN2 NeuronCores, data-parallel over batch.

Per example b:
    hs   = H[b] @ W_h + (s[b] @ W_s + b_s)        # [T, A]
    e    = tanh(hs) @ v                            # [T]
    a    = softmax(e)  (mask is all-ones)          # [T]
    ctx  = a @ H[b]                                # [ENC]

Device strategy (per core, 4 examples):
  - H is uploaded pre-transposed per example (HT[b] = H[b].T, [ENC, T]) so the
    ENC contraction lands on SBUF partitions with contiguous DMA.
  - Score matmul: lhsT = W_h chunk [128e, 128a], rhs = HT chunk [128e, 512t],
    float32r (single-pass PE mode, 1 cyc/row) accumulated over 8 e-chunks.
  - tanh fused with the decoder-projection bias d[a] via ScalarE activation.
  - e-scores: lhsT = V (v replicated across 128 columns) so the PE emits
    e broadcast across all 128 partitions in one shot: out[m,t] = sum_a v[a]*tanh[a,t].
  - softmax without max-subtraction: |e| <= ||v||_1 ~ 13, exp is safe in f32.
    exp on ScalarE with accum_out giving the running denominator l.
  - ctx: contraction over t is the free dim of HT tiles -> VectorE
    tensor_tensor_reduce(HT_tile * w) accumulated per (e-chunk, t-chunk).
  - Final 1/l scaling on-chip; outputs ctx [4,1024] and a [4,2048] per core.
"""

import os
import sys

import numpy as np

if "/opt/trn_rl_repo" not in sys.path:
    sys.path.insert(0, "/opt/trn_rl_repo")

import concourse.bacc as bacc
import concourse.bass as bass
import concourse.mybir as mybir
import concourse.tile as tile
from concourse.bass_utils import run_bass_kernel_spmd

B, T, ENC, DEC, A = 32, 2048, 1024, 1024, 256
NCORES = 8
BS = B // NCORES            # examples per core
TCH = 512                   # t-chunk (moving free dim)
NTCH = T // TCH
NEC = ENC // 128            # contraction chunks
NAC = A // 128              # attention-dim chunks
F32 = mybir.dt.float32
F32R = mybir.dt.float32r

_built = {}
LAST_RESULT = None


def _build(use_mask: bool) -> bass.Bass:
    nc = bacc.Bacc()
    ht_d = nc.declare_dram_parameter("ht", [BS, ENC, T], F32R, isOutput=False)
    st_d = nc.declare_dram_parameter("st", [DEC, BS], F32R, isOutput=False)
    wh_d = nc.declare_dram_parameter("wh", [ENC, A], F32R, isOutput=False)
    ws_d = nc.declare_dram_parameter("ws", [DEC, A], F32R, isOutput=False)
    bs_d = nc.declare_dram_parameter("bs", [A], F32, isOutput=False)
    vt_d = nc.declare_dram_parameter("vt", [NAC, 128, 128], F32R, isOutput=False)
    if use_mask:
        nb_d = nc.declare_dram_parameter("nb", [BS, T], F32R, isOutput=False)
    ctx_d = nc.declare_dram_parameter("ctx", [BS, ENC], F32, isOutput=True)
    att_d = nc.declare_dram_parameter("att", [BS, T], F32, isOutput=True)

    TANH = mybir.ActivationFunctionType.Tanh
    EXP = mybir.ActivationFunctionType.Exp
    IDENT = mybir.ActivationFunctionType.Identity
    ADD = mybir.AluOpType.add
    MULT = mybir.AluOpType.mult
    AXX = mybir.AxisListType.X

    with tile.TileContext(nc) as tc:
        with (
            tc.tile_pool(name="const", bufs=1) as cpool,
            tc.tile_pool(name="hbuf", bufs=3) as hpool,
            tc.tile_pool(name="work", bufs=3) as wpool,
            tc.tile_pool(name="scr", bufs=2) as spool,
            tc.tile_pool(name="psA", bufs=4, space="PSUM") as psA,
            tc.tile_pool(name="psB", bufs=2, space="PSUM") as psB,
        ):
            # ---- constants / parameters ----
            wh_sb = cpool.tile([128, NEC, A], F32R)
            nc.sync.dma_start(wh_sb[:], wh_d[:].rearrange("(c p) a -> p c a", p=128))
            ws_sb = cpool.tile([128, NEC, A], F32R)
            nc.sync.dma_start(ws_sb[:], ws_d[:].rearrange("(c p) a -> p c a", p=128))
            st_sb = cpool.tile([128, DEC // 128, BS], F32R)
            nc.sync.dma_start(st_sb[:], st_d[:].rearrange("(c p) b -> p c b", p=128))
            bs_sb = cpool.tile([128, NAC], F32)
            nc.sync.dma_start(bs_sb[:], bs_d[:].rearrange("(c p) -> p c", p=128))
            v_sb = cpool.tile([128, NAC, 128], F32R)
            nc.sync.dma_start(v_sb[:], vt_d[:].rearrange("c p m -> p c m"))
            if use_mask:
                nb_sb = cpool.tile([1, BS, T], F32R)
                nc.sync.dma_start(nb_sb[:], nb_d[:])
                ones1 = cpool.tile([1, 128], F32R)
                nc.vector.memset(ones1[:], 1.0)

            # persistent accumulators
            d_sb = cpool.tile([128, NAC, BS], F32)
            lacc = cpool.tile([128, BS, NTCH], F32)
            ctxp = cpool.tile([128, BS, NEC, NTCH], F32)
            ctxu = cpool.tile([128, BS, NEC], F32)
            ctxf = cpool.tile([128, BS, NEC], F32)
            ltot = cpool.tile([128, BS], F32)
            rlt = cpool.tile([128, BS], F32)
            araw = cpool.tile([1, BS, T], F32)

            # ---- d = s @ W_s + b_s  (per a-chunk, all examples at once) ----
            for ac in range(NAC):
                pd = psB.tile([128, BS], F32, tag="d")
                for ec in range(DEC // 128):
                    nc.tensor.matmul(
                        pd[:],
                        ws_sb[:, ec, ac * 128:(ac + 1) * 128],
                        st_sb[:, ec, :],
                        start=(ec == 0),
                        stop=(ec == DEC // 128 - 1),
                    )
                nc.vector.tensor_scalar_add(d_sb[:, ac, :], pd[:], bs_sb[:, ac:ac + 1])

            # ---- main pipeline ----
            for b in range(BS):
                for i in range(NTCH):
                    t0 = i * TCH
                    ht_t = hpool.tile([128, NEC, TCH], F32R, tag="ht")
                    nc.sync.dma_start(
                        ht_t[:],
                        ht_d[b].rearrange("(c p) t -> p c t", p=128)[:, :, t0:t0 + TCH],
                    )
                    th_t = wpool.tile([128, NAC, TCH], F32R, tag="th")
                    for ac in range(NAC):
                        ph = psA.tile([128, TCH], F32, tag="hs")
                        for ec in range(NEC):
                            nc.tensor.matmul(
                                ph[:],
                                wh_sb[:, ec, ac * 128:(ac + 1) * 128],
                                ht_t[:, ec, :],
                                start=(ec == 0),
                                stop=(ec == NEC - 1),
                            )
                        nc.scalar.activation(
                            th_t[:, ac, :], ph[:], TANH, bias=d_sb[:, ac, b:b + 1]
                        )
                    pe_ = psB.tile([128, TCH], F32, tag="e")
                    last_mm = NAC - 1 if not use_mask else NAC
                    for ac in range(NAC):
                        nc.tensor.matmul(
                            pe_[:],
                            v_sb[:, ac, :],
                            th_t[:, ac, :],
                            start=(ac == 0),
                            stop=(ac == last_mm),
                        )
                    if use_mask:
                        nc.tensor.matmul(
                            pe_[:],
                            ones1[:],
                            nb_sb[0:1, b, t0:t0 + TCH],
                            start=False,
                            stop=True,
                        )
                    w_t = wpool.tile([128, TCH], F32, tag="w")
                    nc.scalar.activation(
                        w_t[:], pe_[:], EXP, accum_out=lacc[:, b, i:i + 1]
                    )
                    nc.scalar.activation(araw[0:1, b, t0:t0 + TCH], pe_[0:1, :], EXP)
                    for ec in range(NEC):
                        scr = spool.tile([128, TCH], F32, tag="scr")
                        nc.vector.tensor_mul(
                            scr[:], ht_t[:, ec, :].bitcast(F32), w_t[:]
                        )
                        nc.scalar.activation(
                            scr[:], scr[:], IDENT,
                            accum_out=ctxp[:, b, ec, i:i + 1],
                        )
                # ---- per-example epilogue ----
                nc.vector.tensor_reduce(ltot[:, b:b + 1], lacc[:, b, :], axis=AXX, op=ADD)
                nc.vector.reciprocal(rlt[:, b:b + 1], ltot[:, b:b + 1])
                nc.vector.tensor_reduce(ctxu[:, b, :], ctxp[:, b, :, :], axis=AXX, op=ADD)
                nc.vector.tensor_scalar_mul(ctxf[:, b, :], ctxu[:, b, :], rlt[:, b:b + 1])
                nc.sync.dma_start(ctx_d[b].rearrange("(c p) -> p c", p=128), ctxf[:, b, :])
                af = wpool.tile([1, T], F32, tag="af")
                nc.vector.tensor_scalar_mul(af[:], araw[0:1, b, :], rlt[0:1, b:b + 1])
                nc.sync.dma_start(att_d[b], af[:])
    nc.finalize()
    return nc


def kernel(H, s, mask, W_h, W_s, b_s, v):
    global LAST_RESULT
    H = np.ascontiguousarray(np.asarray(H, dtype=np.float32))
    s = np.asarray(s, dtype=np.float32)
    mask = np.asarray(mask)
    W_h = np.ascontiguousarray(np.asarray(W_h, dtype=np.float32))
    W_s = np.ascontiguousarray(np.asarray(W_s, dtype=np.float32))
    b_s = np.ascontiguousarray(np.asarray(b_s, dtype=np.float32))
    v = np.asarray(v, dtype=np.float32)
    assert H.shape == (B, T, ENC) and s.shape == (B, DEC)

    use_mask = not bool(mask.all())
    if use_mask not in _built:
        _built[use_mask] = _build(use_mask)
    nc = _built[use_mask]

    vt = np.ascontiguousarray(
        np.repeat(v.reshape(NAC, 128, 1), 128, axis=2).astype(np.float32)
    )
    nb_full = None
    if use_mask:
        nb_full = np.where(mask, 0.0, -1e30).astype(np.float32)

    in_maps = []
    for c in range(NCORES):
        sl = slice(c * BS, (c + 1) * BS)
        m = {
            "ht": np.ascontiguousarray(H[sl].transpose(0, 2, 1)),
            "st": np.ascontiguousarray(s[sl].T),
            "wh": W_h,
            "ws": W_s,
            "bs": b_s,
            "vt": vt,
        }
        if use_mask:
            m["nb"] = np.ascontiguousarray(nb_full[sl])
        in_maps.append(m)

    res = run_bass_kernel_spmd(nc, in_maps, core_ids=list(range(NCORES)))
    LAST_RESULT = res

    ctx = np.concatenate([np.asarray(r["ctx"]) for r in res.results], axis=0)
    att = np.concatenate([np.asarray(r["att"]) for r in res.results], axis=0)
    return ctx.astype(np.float32), att.astype(np.float32)


if __name__ == "__main__":
    rng = np.random.default_rng(0)
    H = rng.standard_normal((B, T, ENC), dtype=np.float32)
    s = rng.standard_normal((B, DEC), dtype=np.float32)
    mask = np.ones((B, T), dtype=bool)
    W_h = rng.standard_normal((ENC, A), dtype=np.float32) / np.sqrt(ENC)
    W_s = rng.standard_normal((DEC, A), dtype=np.float32) / np.sqrt(DEC)
    b_s = np.zeros((A,), dtype=np.float32)
    v = rng.standard_normal((A,), dtype=np.float32) / np.sqrt(A)
    ctx, att = kernel(H=H, s=s, mask=mask, W_h=W_h, W_s=W_s, b_s=b_s, v=v)
    print(ctx.shape, att.shape, float(att.sum(axis=1).mean()))


# revision 9
# speedup vs baseline: 1.6219x; 1.6219x over previous
"""Bahdanau attention on 8 TRN2 NeuronCores, data-parallel over batch.

Per example b:
    hs   = H[b] @ W_h + (s[b] @ W_s + b_s)        # [T, A]
    e    = tanh(hs) @ v                            # [T]
    a    = softmax(e)  (mask is all-ones)          # [T]
    ctx  = a @ H[b]                                # [ENC]

Device strategy (per core, 4 examples, bf16 compute / f32 accumulate):
  - H is uploaded pre-transposed per example (HT[b] = H[b].T, [ENC, T]) in
    bf16, so the ENC contraction lands on SBUF partitions with contiguous
    4 KB DMA runs, and HBM traffic is halved.
  - Score matmul: lhsT = W_h chunk [128e, 128a] bf16, rhs = HT chunk
    [128e, 512t] bf16, accumulated over 8 e-chunks into f32 PSUM.
  - tanh fused with the decoder-projection bias d[a] via ScalarE activation.
  - e-scores: lhsT = V (v replicated across 128 columns) so the PE emits
    e broadcast across all 128 partitions: out[m,t] = sum_a v[a]*tanh[a,t].
  - softmax without max-subtraction: |e| <= ||v||_1 ~ 13, exp is safe in f32.
  - ctx: contraction over t is the free dim of HT tiles -> one VectorE
    tensor_mul (bf16 2x mode, w broadcast via a stride-0 AP) + per-e-chunk
    free-dim reduces split between ScalarE (Identity+accum_out) and VectorE
    (tensor_reduce), writing unnormalized partials.
  - Normalization (1/l) and the final partial sums happen on the host:
    device outputs are exp(e) rows and per-(chunk, e-chunk) ctx partials.
"""

import os
import sys

import numpy as np

if "/opt/trn_rl_repo" not in sys.path:
    sys.path.insert(0, "/opt/trn_rl_repo")

import ml_dtypes

import concourse.bacc as bacc
import concourse.bass as bass
import concourse.mybir as mybir
import concourse.tile as tile
from concourse.bass_utils import run_bass_kernel_spmd

B, T, ENC, DEC, A = 32, 2048, 1024, 1024, 256
NCORES = 8
BS = B // NCORES            # examples per core
TCH = 512                   # t-chunk (moving free dim)
NTCH = T // TCH
NEC = ENC // 128            # contraction chunks
NAC = A // 128              # attention-dim chunks
N_ACT_RED = 5               # e-chunk reduces on ScalarE (rest on VectorE)
F32 = mybir.dt.float32
BF16 = mybir.dt.bfloat16
NPBF16 = ml_dtypes.bfloat16

_built = {}
LAST_RESULT = None


def _build(use_mask: bool) -> bass.Bass:
    nc = bacc.Bacc()
    ht_d = nc.declare_dram_parameter("ht", [BS, ENC, T], BF16, isOutput=False)
    st_d = nc.declare_dram_parameter("st", [DEC, BS], BF16, isOutput=False)
    wh_d = nc.declare_dram_parameter("wh", [ENC, A], BF16, isOutput=False)
    ws_d = nc.declare_dram_parameter("ws", [DEC, A], BF16, isOutput=False)
    bs_d = nc.declare_dram_parameter("bs", [A], F32, isOutput=False)
    vt_d = nc.declare_dram_parameter("vt", [NAC, 128, 128], BF16, isOutput=False)
    if use_mask:
        nb_d = nc.declare_dram_parameter("nb", [BS, T], BF16, isOutput=False)
    ctxp_d = nc.declare_dram_parameter("ctxp", [128, BS, NEC], F32, isOutput=True)
    araw_d = nc.declare_dram_parameter("araw", [BS, T], BF16, isOutput=True)

    TANH = mybir.ActivationFunctionType.Tanh
    EXP = mybir.ActivationFunctionType.Exp
    IDENT = mybir.ActivationFunctionType.Identity
    ADD = mybir.AluOpType.add
    AXX = mybir.AxisListType.X

    with tile.TileContext(nc) as tc:
        with (
            tc.tile_pool(name="const", bufs=1) as cpool,
            tc.tile_pool(name="hbuf", bufs=2) as hpool,
            tc.tile_pool(name="work", bufs=3) as wpool,
            tc.tile_pool(name="scr", bufs=2) as spool,
            tc.tile_pool(name="psA", bufs=4, space="PSUM") as psA,
            tc.tile_pool(name="psB", bufs=2, space="PSUM") as psB,
        ):
            # ---- constants / parameters ----
            wh_sb = cpool.tile([128, NEC, A], BF16)
            nc.sync.dma_start(wh_sb[:], wh_d[:].rearrange("(c p) a -> p c a", p=128))
            ws_sb = cpool.tile([128, NEC, A], BF16)
            nc.sync.dma_start(ws_sb[:], ws_d[:].rearrange("(c p) a -> p c a", p=128))
            st_sb = cpool.tile([128, DEC // 128, BS], BF16)
            nc.sync.dma_start(st_sb[:], st_d[:].rearrange("(c p) b -> p c b", p=128))
            bs_sb = cpool.tile([128, NAC], F32)
            nc.sync.dma_start(bs_sb[:], bs_d[:].rearrange("(c p) -> p c", p=128))
            v_sb = cpool.tile([128, NAC, 128], BF16)
            nc.sync.dma_start(v_sb[:], vt_d[:].rearrange("c p m -> p c m"))
            if use_mask:
                nb_sb = cpool.tile([1, BS, T], BF16)
                nc.sync.dma_start(nb_sb[:], nb_d[:])
                ones1 = cpool.tile([1, 128], BF16)
                nc.vector.memset(ones1[:], 1.0)

            # persistent accumulators / outputs
            d_sb = cpool.tile([128, NAC, BS], F32)
            ctxp = cpool.tile([128, BS, NEC], F32)
            araw = cpool.tile([1, BS, T], BF16)

            # ---- d = s @ W_s + b_s  (per a-chunk, all examples at once) ----
            for ac in range(NAC):
                pd = psB.tile([128, BS], F32, tag="d")
                for ec in range(DEC // 128):
                    nc.tensor.matmul(
                        pd[:],
                        ws_sb[:, ec, ac * 128:(ac + 1) * 128],
                        st_sb[:, ec, :],
                        start=(ec == 0),
                        stop=(ec == DEC // 128 - 1),
                    )
                nc.vector.tensor_scalar_add(d_sb[:, ac, :], pd[:], bs_sb[:, ac:ac + 1])

            # ---- main pipeline ----
            for b in range(BS):
                # whole example in one DMA: 4 KB contiguous runs per (p, ec)
                ht_t = hpool.tile([128, NEC, T], BF16, tag="ht")
                nc.sync.dma_start(
                    ht_t[:], ht_d[b].rearrange("(c p) t -> p c t", p=128)
                )
                w_ex = wpool.tile([128, 1, T], BF16, tag="w")
                for i in range(NTCH):
                    t0 = i * TCH
                    th_t = wpool.tile([128, NAC, TCH], BF16, tag="th")
                    for ac in range(NAC):
                        ph = psA.tile([128, TCH], F32, tag="hs")
                        for ec in range(NEC):
                            nc.tensor.matmul(
                                ph[:],
                                wh_sb[:, ec, ac * 128:(ac + 1) * 128],
                                ht_t[:, ec, t0:t0 + TCH],
                                start=(ec == 0),
                                stop=(ec == NEC - 1),
                            )
                        nc.scalar.activation(
                            th_t[:, ac, :], ph[:], TANH, bias=d_sb[:, ac, b:b + 1]
                        )
                    pe_ = psB.tile([128, TCH], F32, tag="e")
                    last_mm = NAC - 1 if not use_mask else NAC
                    for ac in range(NAC):
                        nc.tensor.matmul(
                            pe_[:],
                            v_sb[:, ac, :],
                            th_t[:, ac, :],
                            start=(ac == 0),
                            stop=(ac == last_mm),
                        )
                    if use_mask:
                        nc.tensor.matmul(
                            pe_[:],
                            ones1[:],
                            nb_sb[0:1, b, t0:t0 + TCH],
                            start=False,
                            stop=True,
                        )
                    nc.scalar.activation(w_ex[:, 0, t0:t0 + TCH], pe_[:], EXP)
                # ---- per-example ctx partials ----
                nc.vector.tensor_copy(araw[0:1, b, :], w_ex[0:1, 0, :])
                scr = spool.tile([128, NEC, T], BF16, tag="scr")
                nc.vector.tensor_mul(
                    scr[:],
                    ht_t[:],
                    w_ex[:].broadcast_to((128, NEC, T)),
                )
                for ec in range(NEC):
                    if ec < N_ACT_RED:
                        nc.scalar.activation(
                            scr[:, ec, :], scr[:, ec, :], IDENT,
                            accum_out=ctxp[:, b, ec:ec + 1],
                        )
                    else:
                        nc.vector.tensor_reduce(
                            ctxp[:, b, ec:ec + 1], scr[:, ec, :],
                            axis=AXX, op=ADD,
                        )
            nc.sync.dma_start(ctxp_d[:], ctxp[:])
            nc.sync.dma_start(araw_d[:], araw[0:1, :, :])
    nc.finalize()
    return nc


def kernel(H, s, mask, W_h, W_s, b_s, v):
    global LAST_RESULT
    H = np.asarray(H, dtype=np.float32)
    s = np.asarray(s, dtype=np.float32)
    mask = np.asarray(mask)
    W_h = np.asarray(W_h, dtype=np.float32)
    W_s = np.asarray(W_s, dtype=np.float32)
    b_s = np.ascontiguousarray(np.asarray(b_s, dtype=np.float32))
    v = np.asarray(v, dtype=np.float32)
    assert H.shape == (B, T, ENC) and s.shape == (B, DEC)

    use_mask = not bool(mask.all())
    if use_mask not in _built:
        _built[use_mask] = _build(use_mask)
    nc = _built[use_mask]

    wh_b = np.ascontiguousarray(W_h.astype(NPBF16))
    ws_b = np.ascontiguousarray(W_s.astype(NPBF16))
    vt = np.ascontiguousarray(
        np.repeat(v.reshape(NAC, 128, 1), 128, axis=2).astype(NPBF16)
    )
    nb_full = None
    if use_mask:
        nb_full = np.where(mask, 0.0, -1e30).astype(NPBF16)

    in_maps = []
    for c in range(NCORES):
        sl = slice(c * BS, (c + 1) * BS)
        m = {
            "ht": np.ascontiguousarray(
                H[sl].transpose(0, 2, 1).astype(NPBF16)
            ),
            "st": np.ascontiguousarray(s[sl].T.astype(NPBF16)),
            "wh": wh_b,
            "ws": ws_b,
            "bs": b_s,
            "vt": vt,
        }
        if use_mask:
            m["nb"] = np.ascontiguousarray(nb_full[sl])
        in_maps.append(m)

    res = run_bass_kernel_spmd(nc, in_maps, core_ids=list(range(NCORES)))
    LAST_RESULT = res

    ctx = np.empty((B, ENC), dtype=np.float32)
    att = np.empty((B, T), dtype=np.float32)
    for c in range(NCORES):
        r = res.results[c]
        araw = np.asarray(r["araw"], dtype=np.float32)          # [BS, T]
        ctx_u = np.asarray(r["ctxp"], dtype=np.float32)         # [128,BS,NEC]
        l = araw.sum(axis=1)                                    # [BS]
        # ctx[b, ec*128 + p] = ctx_u[p, b, ec]
        ctx[c * BS:(c + 1) * BS] = (
            ctx_u.transpose(1, 2, 0).reshape(BS, ENC) / l[:, None]
        )
        att[c * BS:(c + 1) * BS] = araw / l[:, None]
    return ctx, att


if __name__ == "__main__":
    rng = np.random.default_rng(0)
    H = rng.standard_normal((B, T, ENC), dtype=np.float32)
    s = rng.standard_normal((B, DEC), dtype=np.float32)
    mask = np.ones((B, T), dtype=bool)
    W_h = rng.standard_normal((ENC, A), dtype=np.float32) / np.sqrt(ENC)
    W_s = rng.standard_normal((DEC, A), dtype=np.float32) / np.sqrt(DEC)
    b_s = np.zeros((A,), dtype=np.float32)
    v = rng.standard_normal((A,), dtype=np.float32) / np.sqrt(A)
    ctx, att = kernel(H=H, s=s, mask=mask, W_h=W_h, W_s=W_s, b_s=b_s, v=v)
    print(ctx.shape, att.shape, float(att.sum(axis=1).mean()))
